# revision 4
# baseline (speedup 1.0000x reference)
"""Expert-parallel MoE kernel for Trainium2 (8 NeuronCores).

Strategy (hardcoded for the nn_MoE problem: H=1024, E=8, top-k=2, I=1408,
shared-I=2816, T=2*2048=4096 tokens, f32 inputs):

- Expert parallel: core r owns routed expert r (dense compute over all T
  tokens, mathematically identical to the reference's dense einsum+combine).
- Shared expert is tensor-parallel: core r owns columns [r*352,(r+1)*352) of
  the shared intermediate dim.
- The gate (softmax top-2) is computed redundantly on every core in fp32 so
  routing decisions match the fp32 reference exactly; each core extracts the
  combine weight of its own expert (its gate matrix is permuted so its own
  expert sits in column 0).
- Each core produces partial = w_e(t)*expert_e(x)(t) + shared_partial(t) for
  all tokens, laid out as [H, T].  A ReduceScatter over the 8 cores sums the
  partials; core r ends up with rows [r*128,(r+1)*128) of y^T.  The host
  concatenates and transposes.
- All big matmuls run in bf16 with f32 PSUM accumulation; the gate runs in
  f32.  Work is split into 8 token chunks of 512 so the per-chunk
  ReduceScatter overlaps with compute of the following chunk.

Layouts put features on the partition axis and tokens on the free axis for
every matmul:
    up:   hg[i, t] = sum_h wg[h, i] * xT[h, t]     (lhsT=wg nat., rhs=xT nat.)
    down: eo[h, t] = sum_i wd[i, h] * act[i, t]    (lhsT=wd nat., rhs=act)
"""

import os
import sys

for _p in ("/opt/trn_rl_repo", "/root/.axon_site/_ro/trn_rl_repo"):
    if os.path.isdir(_p) and _p not in sys.path:
        sys.path.insert(0, _p)

import numpy as np

import concourse.bass as bass
import concourse.mybir as mybir
import concourse.tile as tile
from concourse import bacc
from concourse.bass_utils import run_bass_kernel_spmd

F32 = mybir.dt.float32
BF16 = mybir.dt.bfloat16
BF16_NP = mybir.dt.np(mybir.dt.bfloat16)
AX = mybir.AxisListType
ALU = mybir.AluOpType
ACTF = mybir.ActivationFunctionType

H = 1024          # hidden
E = 8             # experts = cores
I_R = 1408        # routed intermediate
SI = 352          # shared intermediate shard per core (2816 / 8)
N_CORES = 8
KC = H // 128     # 8 contraction chunks
IT_R = I_R // 128  # 11 routed intermediate tiles
SH_TILES = [(0, 0, 128), (1, 128, 128), (2, 256, 96)]  # shared i tiles
NEG_BIG = -1.0e30

LAST_RESULT = None  # BassKernelResults of the most recent run (for profiling)


def build_nc(T=4096, TC=512, trace_sim=False, silu_via_sigmoid=False):
    """Build the SPMD Bass program (identical on all 8 cores).

    silu_via_sigmoid: CoreSim has no Silu LUT; emulate it exactly as
    x*sigmoid(x) (an extra DVE multiply) for simulation runs only.
    """
    n_chunks = T // TC
    n_sub = TC // 128
    nc = bacc.Bacc("TRN2", target_bir_lowering=False, debug=False,
                   num_devices=N_CORES)

    xT = nc.dram_tensor("xT", [H, T], F32, kind="ExternalInput")
    gwT = nc.dram_tensor("gwT", [H, E], F32, kind="ExternalInput")
    ident = nc.dram_tensor("ident", [128, 128], F32, kind="ExternalInput")
    wg = nc.dram_tensor("wg", [H, I_R], BF16, kind="ExternalInput")
    wu = nc.dram_tensor("wu", [H, I_R], BF16, kind="ExternalInput")
    wd = nc.dram_tensor("wd", [I_R, H], BF16, kind="ExternalInput")
    swg = nc.dram_tensor("swg", [H, SI], BF16, kind="ExternalInput")
    swu = nc.dram_tensor("swu", [H, SI], BF16, kind="ExternalInput")
    swd = nc.dram_tensor("swd", [SI, H], BF16, kind="ExternalInput")
    y = nc.dram_tensor("y", [128, T], F32, kind="ExternalOutput")

    rg = [list(range(N_CORES))]

    with tile.TileContext(nc, trace_sim=trace_sim) as tc:
        with (
            tc.tile_pool(name="const", bufs=1) as cpool,
            tc.tile_pool(name="xf", bufs=2) as xfpool,
            tc.tile_pool(name="xb", bufs=2) as xbpool,
            tc.tile_pool(name="gate", bufs=2) as gpool,
            tc.tile_pool(name="actr", bufs=2) as actrpool,
            tc.tile_pool(name="acts", bufs=2) as actspool,
            tc.tile_pool(name="tmp", bufs=3) as tpool,
            tc.tile_pool(name="eo", bufs=3) as eopool,
            tc.tile_pool(name="ps_small", bufs=3, space="PSUM") as ps_small,
            tc.tile_pool(name="ps_up", bufs=3, space="PSUM") as ps_up,
            tc.tile_pool(name="ps_o", bufs=2, space="PSUM") as ps_o,
            tc.tile_pool(name="dram", bufs=2, space="DRAM") as dpool,
        ):
            # ---- resident constants / weights ----
            gw_t = cpool.tile([128, KC, E], F32)
            for k in range(KC):
                nc.sync.dma_start(gw_t[:, k, :], gwT[k * 128:(k + 1) * 128, :])
            id_t = cpool.tile([128, 128], F32)
            nc.sync.dma_start(id_t[:, :], ident[:, :])
            ones = cpool.tile([1, 128], F32)
            nc.vector.memset(ones[:, :], 1.0)

            wg_t = cpool.tile([128, KC, I_R], BF16)
            wu_t = cpool.tile([128, KC, I_R], BF16)
            for k in range(KC):
                nc.sync.dma_start(wg_t[:, k, :], wg[k * 128:(k + 1) * 128, :])
                nc.sync.dma_start(wu_t[:, k, :], wu[k * 128:(k + 1) * 128, :])
            wd_t = cpool.tile([128, IT_R, H], BF16)
            for it in range(IT_R):
                nc.sync.dma_start(wd_t[:, it, :], wd[it * 128:(it + 1) * 128, :])
            swg_t = cpool.tile([128, KC, SI], BF16)
            swu_t = cpool.tile([128, KC, SI], BF16)
            for k in range(KC):
                nc.sync.dma_start(swg_t[:, k, :], swg[k * 128:(k + 1) * 128, :])
                nc.sync.dma_start(swu_t[:, k, :], swu[k * 128:(k + 1) * 128, :])
            swd_t = cpool.tile([128, len(SH_TILES), H], BF16)
            for it, m0, msz in SH_TILES:
                nc.sync.dma_start(swd_t[:msz, it, :], swd[m0:m0 + msz, :])

            for c in range(n_chunks):
                t0 = c * TC
                # ---- load x chunk (f32) and cast to bf16 ----
                xf = xfpool.tile([128, KC, TC], F32)
                for k in range(KC):
                    nc.sync.dma_start(
                        xf[:, k, :], xT[k * 128:(k + 1) * 128, t0:t0 + TC])
                xb = xbpool.tile([128, KC, TC], BF16)
                nc.vector.tensor_copy(xb[:, :, :], xf[:, :, :])

                # ---- gate: fp32 softmax top-2 weight of own expert ----
                wrow = gpool.tile([1, TC], F32)
                for j in range(n_sub):
                    pl = ps_small.tile([128, E], F32, tag="sm")
                    for k in range(KC):
                        nc.tensor.matmul(
                            pl[:, :], xf[:, k, j * 128:(j + 1) * 128],
                            gw_t[:, k, :], start=(k == 0), stop=(k == KC - 1))
                    lg = gpool.tile([128, E], F32)
                    nc.vector.tensor_copy(lg[:, :], pl[:, :])
                    m1 = gpool.tile([128, 1], F32)
                    nc.vector.reduce_max(m1[:, :], lg[:, :], axis=AX.X)
                    negm1 = gpool.tile([128, 1], F32)
                    nc.vector.tensor_scalar_mul(negm1[:, :], m1[:, :], -1.0)
                    eq1 = gpool.tile([128, E], F32)
                    nc.vector.tensor_scalar(
                        eq1[:, :], lg[:, :], m1[:, 0:1], None, op0=ALU.is_equal)
                    masked = gpool.tile([128, E], F32)
                    nc.vector.scalar_tensor_tensor(
                        masked[:, :], eq1[:, :], NEG_BIG, lg[:, :],
                        op0=ALU.mult, op1=ALU.add)
                    m2l = gpool.tile([128, 1], F32)
                    nc.vector.reduce_max(m2l[:, :], masked[:, :], axis=AX.X)
                    s2 = gpool.tile([128, 1], F32)
                    nc.scalar.activation(
                        s2[:, :], m2l[:, :], ACTF.Exp, bias=negm1[:, 0:1])
                    den = gpool.tile([128, 1], F32)
                    nc.vector.tensor_scalar_add(den[:, :], s2[:, :], 1.0)
                    inv = gpool.tile([128, 1], F32)
                    nc.vector.reciprocal(inv[:, :], den[:, :])
                    se = gpool.tile([128, 1], F32)
                    nc.scalar.activation(
                        se[:, :], lg[:, 0:1], ACTF.Exp, bias=negm1[:, 0:1])
                    sel = gpool.tile([128, 1], F32)
                    nc.vector.tensor_scalar(
                        sel[:, :], lg[:, 0:1], m2l[:, 0:1], None, op0=ALU.is_ge)
                    wcol = gpool.tile([128, 1], F32)
                    nc.vector.tensor_scalar(
                        wcol[:, :], se[:, :], sel[:, 0:1], inv[:, 0:1],
                        op0=ALU.mult, op1=ALU.mult)
                    ptr = ps_small.tile([1, 128], F32, tag="sm")
                    nc.tensor.transpose(ptr[:, :], wcol[:, :], id_t[:, :])
                    nc.vector.tensor_copy(wrow[0:1, j * 128:(j + 1) * 128],
                                          ptr[:, :])
                # broadcast w over 128 partitions
                pw = ps_small.tile([128, TC], F32, tag="sm")
                nc.tensor.matmul(pw[:, :], ones[0:1, :], wrow[0:1, :],
                                 start=True, stop=True)
                wb = gpool.tile([128, TC], F32)
                nc.vector.tensor_copy(wb[:, :], pw[:, :])

                # ---- routed expert up-proj + swiglu (scaled by gate w) ----
                actr = actrpool.tile([128, IT_R, TC], BF16)
                for it in range(IT_R):
                    pg = ps_up.tile([128, TC], F32, tag="up")
                    for k in range(KC):
                        nc.tensor.matmul(
                            pg[:, :], wg_t[:, k, it * 128:(it + 1) * 128],
                            xb[:, k, :], start=(k == 0), stop=(k == KC - 1))
                    pu = ps_up.tile([128, TC], F32, tag="up")
                    for k in range(KC):
                        nc.tensor.matmul(
                            pu[:, :], wu_t[:, k, it * 128:(it + 1) * 128],
                            xb[:, k, :], start=(k == 0), stop=(k == KC - 1))
                    sg = tpool.tile([128, TC], F32, tag="sg")
                    if silu_via_sigmoid:
                        nc.scalar.activation(sg[:, :], pg[:, :], ACTF.Sigmoid)
                        nc.vector.tensor_mul(sg[:, :], sg[:, :], pg[:, :])
                    else:
                        nc.scalar.activation(sg[:, :], pg[:, :], ACTF.Silu)
                    tt = tpool.tile([128, TC], F32, tag="tt")
                    nc.vector.tensor_mul(tt[:, :], sg[:, :], pu[:, :])
                    nc.vector.tensor_mul(actr[:, it, :], tt[:, :], wb[:, :])

                # ---- shared expert shard up-proj + swiglu ----
                acts = actspool.tile([128, len(SH_TILES), TC], BF16)
                for it, m0, msz in SH_TILES:
                    pg = ps_up.tile([128, TC], F32, tag="up")
                    for k in range(KC):
                        nc.tensor.matmul(
                            pg[:msz, :], swg_t[:, k, m0:m0 + msz],
                            xb[:, k, :], start=(k == 0), stop=(k == KC - 1))
                    pu = ps_up.tile([128, TC], F32, tag="up")
                    for k in range(KC):
                        nc.tensor.matmul(
                            pu[:msz, :], swu_t[:, k, m0:m0 + msz],
                            xb[:, k, :], start=(k == 0), stop=(k == KC - 1))
                    sg = tpool.tile([128, TC], F32, tag="sg")
                    if silu_via_sigmoid:
                        nc.scalar.activation(sg[:msz, :], pg[:msz, :],
                                             ACTF.Sigmoid)
                        nc.vector.tensor_mul(sg[:msz, :], sg[:msz, :],
                                             pg[:msz, :])
                    else:
                        nc.scalar.activation(sg[:msz, :], pg[:msz, :],
                                             ACTF.Silu)
                    nc.vector.tensor_mul(acts[:msz, it, :], sg[:msz, :],
                                         pu[:msz, :])

                # ---- down-proj (routed + shared into one accumulator) ----
                ccin = dpool.tile([H, TC], F32, tag="ccin")
                for hc in range(KC):
                    h0 = hc * 128
                    po = ps_o.tile([128, TC], F32, tag="o")
                    for it in range(IT_R):
                        nc.tensor.matmul(
                            po[:, :], wd_t[:, it, h0:h0 + 128],
                            actr[:, it, :], start=(it == 0), stop=False)
                    for it, m0, msz in SH_TILES:
                        nc.tensor.matmul(
                            po[:, :], swd_t[:msz, it, h0:h0 + 128],
                            acts[:msz, it, :], start=False,
                            stop=(it == len(SH_TILES) - 1))
                    eo = eopool.tile([128, TC], F32)
                    nc.vector.tensor_copy(eo[:, :], po[:, :])
                    nc.sync.dma_start(ccin[h0:h0 + 128, :], eo[:, :])

                # ---- combine across cores: ReduceScatter this chunk ----
                ccout = dpool.tile([128, TC], F32, tag="ccout")
                nc.gpsimd.collective_compute(
                    "ReduceScatter", ALU.add, replica_groups=rg,
                    ins=[ccin.opt()], outs=[ccout.opt()])
                nc.sync.dma_start(y[:, t0:t0 + TC], ccout[:, :])

    nc.compile()
    return nc


def make_in_maps(x, gate_w, wg, wu, wd, swg, swu, swd, T=4096):
    xT = np.ascontiguousarray(
        x.reshape(-1, H).T).astype(np.float32)[:, :T]
    ident = np.eye(128, dtype=np.float32)
    in_maps = []
    for r in range(N_CORES):
        perm = [r] + [e for e in range(E) if e != r]
        in_maps.append({
            "xT": xT,
            "gwT": np.ascontiguousarray(gate_w[perm].T.astype(np.float32)),
            "ident": ident,
            "wg": np.ascontiguousarray(wg[r]).astype(BF16_NP),
            "wu": np.ascontiguousarray(wu[r]).astype(BF16_NP),
            "wd": np.ascontiguousarray(wd[r]).astype(BF16_NP),
            "swg": np.ascontiguousarray(swg[:, r * SI:(r + 1) * SI]).astype(BF16_NP),
            "swu": np.ascontiguousarray(swu[:, r * SI:(r + 1) * SI]).astype(BF16_NP),
            "swd": np.ascontiguousarray(swd[r * SI:(r + 1) * SI, :]).astype(BF16_NP),
        })
    return in_maps


_NC_CACHE = {}


def kernel(x, gate_w, wg, wu, wd, swg, swu, swd):
    global LAST_RESULT
    x = np.asarray(x)
    B, S, _ = x.shape
    T = B * S
    if T not in _NC_CACHE:
        _NC_CACHE[T] = build_nc(T=T)
    nc = _NC_CACHE[T]
    in_maps = make_in_maps(
        np.asarray(x, np.float32), np.asarray(gate_w, np.float32),
        np.asarray(wg, np.float32), np.asarray(wu, np.float32),
        np.asarray(wd, np.float32), np.asarray(swg, np.float32),
        np.asarray(swu, np.float32), np.asarray(swd, np.float32), T=T)
    res = run_bass_kernel_spmd(nc, in_maps, core_ids=list(range(N_CORES)))
    LAST_RESULT = res
    yT = np.concatenate([res.results[r]["y"] for r in range(N_CORES)], axis=0)
    return np.ascontiguousarray(yT.T).reshape(B, S, H).astype(np.float32)


# revision 10
# speedup vs baseline: 1.0302x; 1.0302x over previous
"""Expert-parallel MoE kernel for Trainium2 (8 NeuronCores).

Strategy (hardcoded for the nn_MoE problem: H=1024, E=8, top-k=2, I=1408,
shared-I=2816, T=2*2048=4096 tokens, f32 inputs):

- Expert parallel: core r owns routed expert r (dense compute over all T
  tokens, mathematically identical to the reference's dense einsum+combine).
- Shared expert is tensor-parallel: core r owns columns [r*352,(r+1)*352) of
  the shared intermediate dim.
- The gate (softmax top-2) is computed redundantly on every core in fp32 so
  routing decisions match the fp32 reference exactly; each core extracts the
  combine weight of its own expert (its gate matrix is permuted so its own
  expert sits in column 0).
- Each core produces partial = w_e(t)*expert_e(x)(t) + shared_partial(t) for
  all tokens, laid out as [H, T].  A ReduceScatter over the 8 cores sums the
  partials; core r ends up with rows [r*128,(r+1)*128) of y^T.  The host
  concatenates and transposes.
- All big matmuls run in bf16 with f32 PSUM accumulation; the gate runs in
  f32.  Work is split into 8 token chunks of 512 so the per-chunk
  ReduceScatter overlaps with compute of the following chunk.

Layouts put features on the partition axis and tokens on the free axis for
every matmul:
    up:   hg[i, t] = sum_h wg[h, i] * xT[h, t]     (lhsT=wg nat., rhs=xT nat.)
    down: eo[h, t] = sum_i wd[i, h] * act[i, t]    (lhsT=wd nat., rhs=act)
"""

import os
import sys

for _p in ("/opt/trn_rl_repo", "/root/.axon_site/_ro/trn_rl_repo"):
    if os.path.isdir(_p) and _p not in sys.path:
        sys.path.insert(0, _p)

import numpy as np

import concourse.bass as bass
import concourse.mybir as mybir
import concourse.tile as tile
from concourse import bacc
from concourse.bass_utils import run_bass_kernel_spmd

F32 = mybir.dt.float32
BF16 = mybir.dt.bfloat16
BF16_NP = mybir.dt.np(mybir.dt.bfloat16)
AX = mybir.AxisListType
ALU = mybir.AluOpType
ACTF = mybir.ActivationFunctionType

H = 1024          # hidden
E = 8             # experts = cores
I_R = 1408        # routed intermediate
SI = 352          # shared intermediate shard per core (2816 / 8)
N_CORES = 8
KC = H // 128     # 8 contraction chunks
IT_R = I_R // 128  # 11 routed intermediate tiles
SH_TILES = [(0, 0, 128), (1, 128, 128), (2, 256, 96)]  # shared i tiles
NEG_BIG = -1.0e30

LAST_RESULT = None  # BassKernelResults of the most recent run (for profiling)


def build_nc(T=4096, TC=512, trace_sim=False, silu_via_sigmoid=False):
    """Build the SPMD Bass program (identical on all 8 cores).

    silu_via_sigmoid: CoreSim has no Silu LUT; emulate it exactly as
    x*sigmoid(x) (an extra DVE multiply) for simulation runs only.
    """
    n_chunks = T // TC
    n_sub = TC // 128
    nc = bacc.Bacc("TRN2", target_bir_lowering=False, debug=False,
                   num_devices=N_CORES)

    xT = nc.dram_tensor("xT", [H, T], F32, kind="ExternalInput")
    gwT = nc.dram_tensor("gwT", [H, E], F32, kind="ExternalInput")
    ident = nc.dram_tensor("ident", [128, 128], F32, kind="ExternalInput")
    wg = nc.dram_tensor("wg", [H, I_R], BF16, kind="ExternalInput")
    wu = nc.dram_tensor("wu", [H, I_R], BF16, kind="ExternalInput")
    wd = nc.dram_tensor("wd", [I_R, H], BF16, kind="ExternalInput")
    swg = nc.dram_tensor("swg", [H, SI], BF16, kind="ExternalInput")
    swu = nc.dram_tensor("swu", [H, SI], BF16, kind="ExternalInput")
    swd = nc.dram_tensor("swd", [SI, H], BF16, kind="ExternalInput")
    y = nc.dram_tensor("y", [128, T], F32, kind="ExternalOutput")

    rg = [list(range(N_CORES))]

    with tile.TileContext(nc, trace_sim=trace_sim) as tc:
        with (
            tc.tile_pool(name="const", bufs=1) as cpool,
            tc.tile_pool(name="xf", bufs=2) as xfpool,
            tc.tile_pool(name="xb", bufs=2) as xbpool,
            tc.tile_pool(name="gate", bufs=2) as gpool,
            tc.tile_pool(name="actr", bufs=2) as actrpool,
            tc.tile_pool(name="acts", bufs=2) as actspool,
            tc.tile_pool(name="tmp", bufs=3) as tpool,
            tc.tile_pool(name="eo", bufs=3) as eopool,
            tc.tile_pool(name="ps_small", bufs=3, space="PSUM") as ps_small,
            tc.tile_pool(name="ps_up", bufs=3, space="PSUM") as ps_up,
            tc.tile_pool(name="ps_o", bufs=2, space="PSUM") as ps_o,
            tc.tile_pool(name="dram", bufs=2, space="DRAM") as dpool,
        ):
            # ---- chunk-0 x + gate weights FIRST so PE starts early ----
            xf0 = xfpool.tile([128, KC, TC], F32, tag="xf")
            for k in range(KC):
                nc.sync.dma_start(xf0[:, k, :], xT[k * 128:(k + 1) * 128, 0:TC])
            gw_t = cpool.tile([128, KC, E], F32)
            for k in range(KC):
                nc.sync.dma_start(gw_t[:, k, :], gwT[k * 128:(k + 1) * 128, :])
            id_t = cpool.tile([128, 128], F32)
            nc.sync.dma_start(id_t[:, :], ident[:, :])
            ones = cpool.tile([1, 128], F32)
            nc.vector.memset(ones[:, :], 1.0)

            # ---- weights, split per contraction chunk so the first
            # up-proj matmuls only wait for their own slice ----
            wg_ks, wu_ks = [], []
            for k in range(KC):
                wgk = cpool.tile([128, I_R], BF16, tag=f"wg{k}")
                nc.sync.dma_start(wgk[:, :], wg[k * 128:(k + 1) * 128, :])
                wuk = cpool.tile([128, I_R], BF16, tag=f"wu{k}")
                nc.sync.dma_start(wuk[:, :], wu[k * 128:(k + 1) * 128, :])
                wg_ks.append(wgk)
                wu_ks.append(wuk)
            swg_ks, swu_ks = [], []
            for k in range(KC):
                sgk = cpool.tile([128, SI], BF16, tag=f"sg{k}")
                nc.sync.dma_start(sgk[:, :], swg[k * 128:(k + 1) * 128, :])
                suk = cpool.tile([128, SI], BF16, tag=f"su{k}")
                nc.sync.dma_start(suk[:, :], swu[k * 128:(k + 1) * 128, :])
                swg_ks.append(sgk)
                swu_ks.append(suk)
            wd_ts = []
            for it in range(IT_R):
                wdt = cpool.tile([128, H], BF16, tag=f"wd{it}")
                nc.sync.dma_start(wdt[:, :], wd[it * 128:(it + 1) * 128, :])
                wd_ts.append(wdt)
            swd_ts = []
            for it, m0, msz in SH_TILES:
                sdt = cpool.tile([128, H], BF16, tag=f"sd{it}")
                nc.sync.dma_start(sdt[:msz, :], swd[m0:m0 + msz, :])
                swd_ts.append(sdt)

            for c in range(n_chunks):
                t0 = c * TC
                # ---- load x chunk (f32) and cast to bf16 ----
                if c == 0:
                    xf = xf0
                else:
                    xf = xfpool.tile([128, KC, TC], F32, tag="xf")
                    for k in range(KC):
                        nc.sync.dma_start(
                            xf[:, k, :], xT[k * 128:(k + 1) * 128, t0:t0 + TC])
                xb = xbpool.tile([128, KC, TC], BF16)
                nc.vector.tensor_copy(xb[:, :, :], xf[:, :, :])

                # ---- gate: fp32 softmax top-2 weight of own expert ----
                wrow = gpool.tile([1, TC], F32)
                for j in range(n_sub):
                    pl = ps_small.tile([128, E], F32, tag="sm")
                    for k in range(KC):
                        nc.tensor.matmul(
                            pl[:, :], xf[:, k, j * 128:(j + 1) * 128],
                            gw_t[:, k, :], start=(k == 0), stop=(k == KC - 1))
                    lg = gpool.tile([128, E], F32)
                    nc.vector.tensor_copy(lg[:, :], pl[:, :])
                    m1 = gpool.tile([128, 1], F32)
                    nc.vector.reduce_max(m1[:, :], lg[:, :], axis=AX.X)
                    eq1 = gpool.tile([128, E], F32)
                    nc.vector.tensor_scalar(
                        eq1[:, :], lg[:, :], m1[:, 0:1], None, op0=ALU.is_equal)
                    masked = gpool.tile([128, E], F32)
                    nc.vector.scalar_tensor_tensor(
                        masked[:, :], eq1[:, :], NEG_BIG, lg[:, :],
                        op0=ALU.mult, op1=ALU.add)
                    m2l = gpool.tile([128, 1], F32)
                    nc.vector.reduce_max(m2l[:, :], masked[:, :], axis=AX.X)
                    # top-2 softmax weight of expert 0 (this core's expert):
                    # w = 1[l_e >= m2l] * sigmoid(2*l_e - m1 - m2l)
                    # (for e in top-2, the other top-2 logit is m1+m2l-l_e)
                    arg = gpool.tile([128, 1], F32)
                    nc.vector.tensor_scalar(
                        arg[:, :], lg[:, 0:1], m1[:, 0:1], m2l[:, 0:1],
                        op0=ALU.subtract, op1=ALU.subtract)
                    nc.vector.tensor_add(arg[:, :], arg[:, :], lg[:, 0:1])
                    sig = gpool.tile([128, 1], F32)
                    nc.scalar.activation(sig[:, :], arg[:, :], ACTF.Sigmoid)
                    sel = gpool.tile([128, 1], F32)
                    nc.vector.tensor_scalar(
                        sel[:, :], lg[:, 0:1], m2l[:, 0:1], None, op0=ALU.is_ge)
                    wcol = gpool.tile([128, 1], F32)
                    nc.vector.tensor_mul(wcol[:, :], sig[:, :], sel[:, :])
                    ptr = ps_small.tile([1, 128], F32, tag="sm")
                    nc.tensor.transpose(ptr[:, :], wcol[:, :], id_t[:, :])
                    nc.vector.tensor_copy(wrow[0:1, j * 128:(j + 1) * 128],
                                          ptr[:, :])
                # broadcast w over 128 partitions
                pw = ps_small.tile([128, TC], F32, tag="sm")
                nc.tensor.matmul(pw[:, :], ones[0:1, :], wrow[0:1, :],
                                 start=True, stop=True)
                wb = gpool.tile([128, TC], F32)
                nc.vector.tensor_copy(wb[:, :], pw[:, :])

                # ---- routed expert up-proj + swiglu (scaled by gate w) ----
                actr = actrpool.tile([128, IT_R, TC], BF16)
                for it in range(IT_R):
                    pg = ps_up.tile([128, TC], F32, tag="up")
                    for k in range(KC):
                        nc.tensor.matmul(
                            pg[:, :], wg_ks[k][:, it * 128:(it + 1) * 128],
                            xb[:, k, :], start=(k == 0), stop=(k == KC - 1))
                    pu = ps_up.tile([128, TC], F32, tag="up")
                    for k in range(KC):
                        nc.tensor.matmul(
                            pu[:, :], wu_ks[k][:, it * 128:(it + 1) * 128],
                            xb[:, k, :], start=(k == 0), stop=(k == KC - 1))
                    sg = tpool.tile([128, TC], F32, tag="sg")
                    if silu_via_sigmoid:
                        nc.scalar.activation(sg[:, :], pg[:, :], ACTF.Sigmoid)
                        nc.vector.tensor_mul(sg[:, :], sg[:, :], pg[:, :])
                    else:
                        nc.scalar.activation(sg[:, :], pg[:, :], ACTF.Silu)
                    tt = tpool.tile([128, TC], F32, tag="tt")
                    nc.vector.tensor_mul(tt[:, :], sg[:, :], pu[:, :])
                    nc.vector.tensor_mul(actr[:, it, :], tt[:, :], wb[:, :])

                # ---- shared expert shard up-proj + swiglu ----
                acts = actspool.tile([128, len(SH_TILES), TC], BF16)
                for it, m0, msz in SH_TILES:
                    pg = ps_up.tile([128, TC], F32, tag="up")
                    for k in range(KC):
                        nc.tensor.matmul(
                            pg[:msz, :], swg_ks[k][:, m0:m0 + msz],
                            xb[:, k, :], start=(k == 0), stop=(k == KC - 1))
                    pu = ps_up.tile([128, TC], F32, tag="up")
                    for k in range(KC):
                        nc.tensor.matmul(
                            pu[:msz, :], swu_ks[k][:, m0:m0 + msz],
                            xb[:, k, :], start=(k == 0), stop=(k == KC - 1))
                    sg = tpool.tile([128, TC], F32, tag="sg")
                    if silu_via_sigmoid:
                        nc.scalar.activation(sg[:msz, :], pg[:msz, :],
                                             ACTF.Sigmoid)
                        nc.vector.tensor_mul(sg[:msz, :], sg[:msz, :],
                                             pg[:msz, :])
                    else:
                        nc.scalar.activation(sg[:msz, :], pg[:msz, :],
                                             ACTF.Silu)
                    nc.vector.tensor_mul(acts[:msz, it, :], sg[:msz, :],
                                         pu[:msz, :])

                # ---- down-proj (routed + shared into one accumulator) ----
                ccin = dpool.tile([H, TC], F32, tag="ccin")
                for hc in range(KC):
                    h0 = hc * 128
                    po = ps_o.tile([128, TC], F32, tag="o")
                    for it in range(IT_R):
                        nc.tensor.matmul(
                            po[:, :], wd_ts[it][:, h0:h0 + 128],
                            actr[:, it, :], start=(it == 0), stop=False)
                    for it, m0, msz in SH_TILES:
                        nc.tensor.matmul(
                            po[:, :], swd_ts[it][:msz, h0:h0 + 128],
                            acts[:msz, it, :], start=False,
                            stop=(it == len(SH_TILES) - 1))
                    eo = eopool.tile([128, TC], F32)
                    nc.vector.tensor_copy(eo[:, :], po[:, :])
                    nc.sync.dma_start(ccin[h0:h0 + 128, :], eo[:, :])

                # ---- combine across cores: ReduceScatter this chunk ----
                ccout = dpool.tile([128, TC], F32, tag="ccout")
                nc.gpsimd.collective_compute(
                    "ReduceScatter", ALU.add, replica_groups=rg,
                    ins=[ccin.opt()], outs=[ccout.opt()])
                nc.sync.dma_start(y[:, t0:t0 + TC], ccout[:, :])

    nc.compile()
    return nc


def make_in_maps(x, gate_w, wg, wu, wd, swg, swu, swd, T=4096):
    xT = np.ascontiguousarray(
        x.reshape(-1, H).T).astype(np.float32)[:, :T]
    ident = np.eye(128, dtype=np.float32)
    in_maps = []
    for r in range(N_CORES):
        perm = [r] + [e for e in range(E) if e != r]
        in_maps.append({
            "xT": xT,
            "gwT": np.ascontiguousarray(gate_w[perm].T.astype(np.float32)),
            "ident": ident,
            "wg": np.ascontiguousarray(wg[r]).astype(BF16_NP),
            "wu": np.ascontiguousarray(wu[r]).astype(BF16_NP),
            "wd": np.ascontiguousarray(wd[r]).astype(BF16_NP),
            "swg": np.ascontiguousarray(swg[:, r * SI:(r + 1) * SI]).astype(BF16_NP),
            "swu": np.ascontiguousarray(swu[:, r * SI:(r + 1) * SI]).astype(BF16_NP),
            "swd": np.ascontiguousarray(swd[r * SI:(r + 1) * SI, :]).astype(BF16_NP),
        })
    return in_maps


_NC_CACHE = {}


def kernel(x, gate_w, wg, wu, wd, swg, swu, swd):
    global LAST_RESULT
    x = np.asarray(x)
    B, S, _ = x.shape
    T = B * S
    if T not in _NC_CACHE:
        _NC_CACHE[T] = build_nc(T=T)
    nc = _NC_CACHE[T]
    in_maps = make_in_maps(
        np.asarray(x, np.float32), np.asarray(gate_w, np.float32),
        np.asarray(wg, np.float32), np.asarray(wu, np.float32),
        np.asarray(wd, np.float32), np.asarray(swg, np.float32),
        np.asarray(swu, np.float32), np.asarray(swd, np.float32), T=T)
    res = run_bass_kernel_spmd(nc, in_maps, core_ids=list(range(N_CORES)))
    LAST_RESULT = res
    yT = np.concatenate([res.results[r]["y"] for r in range(N_CORES)], axis=0)
    return np.ascontiguousarray(yT.T).reshape(B, S, H).astype(np.float32)


# revision 18
# speedup vs baseline: 1.0648x; 1.0336x over previous
"""Expert-parallel MoE kernel for Trainium2 (8 NeuronCores).

Strategy (hardcoded for the nn_MoE problem: H=1024, E=8, top-k=2, I=1408,
shared-I=2816, T=2*2048=4096 tokens, f32 inputs):

- Expert parallel: core r owns routed expert r (dense compute over all T
  tokens, mathematically identical to the reference's dense einsum+combine).
- Shared expert is tensor-parallel: core r owns columns [r*352,(r+1)*352) of
  the shared intermediate dim.
- The gate (softmax top-2) is computed redundantly on every core in fp32 so
  routing decisions match the fp32 reference exactly; each core extracts the
  combine weight of its own expert (its gate matrix is permuted so its own
  expert sits in column 0).
- Each core produces partial = w_e(t)*expert_e(x)(t) + shared_partial(t) for
  all tokens, laid out as [H, T].  A ReduceScatter over the 8 cores sums the
  partials; core r ends up with rows [r*128,(r+1)*128) of y^T.  The host
  concatenates and transposes.
- All big matmuls run in bf16 with f32 PSUM accumulation; the gate runs in
  f32.  Work is split into 8 token chunks of 512 so the per-chunk
  ReduceScatter overlaps with compute of the following chunk.

Layouts put features on the partition axis and tokens on the free axis for
every matmul:
    up:   hg[i, t] = sum_h wg[h, i] * xT[h, t]     (lhsT=wg nat., rhs=xT nat.)
    down: eo[h, t] = sum_i wd[i, h] * act[i, t]    (lhsT=wd nat., rhs=act)
"""

import os
import sys

for _p in ("/opt/trn_rl_repo", "/root/.axon_site/_ro/trn_rl_repo"):
    if os.path.isdir(_p) and _p not in sys.path:
        sys.path.insert(0, _p)

import numpy as np

import concourse.bass as bass
import concourse.mybir as mybir
import concourse.tile as tile
from concourse import bacc
from concourse.bass_utils import run_bass_kernel_spmd

F32 = mybir.dt.float32
BF16 = mybir.dt.bfloat16
BF16_NP = mybir.dt.np(mybir.dt.bfloat16)
AX = mybir.AxisListType
ALU = mybir.AluOpType
ACTF = mybir.ActivationFunctionType

H = 1024          # hidden
E = 8             # experts = cores
I_R = 1408        # routed intermediate
SI = 352          # shared intermediate shard per core (2816 / 8)
N_CORES = 8
KC = H // 128     # 8 contraction chunks
IT_R = I_R // 128  # 11 routed intermediate tiles
SH_TILES = [(0, 0, 128), (1, 128, 128), (2, 256, 96)]  # shared i tiles
NEG_BIG = -1.0e30

LAST_RESULT = None  # BassKernelResults of the most recent run (for profiling)


def build_nc(T=4096, TC=512, trace_sim=False, silu_via_sigmoid=False):
    """Build the SPMD Bass program (identical on all 8 cores).

    silu_via_sigmoid: CoreSim has no Silu LUT; emulate it exactly as
    x*sigmoid(x) (an extra DVE multiply) for simulation runs only.
    """
    n_chunks = T // TC
    n_sub = TC // 128
    nc = bacc.Bacc("TRN2", target_bir_lowering=False, debug=False,
                   num_devices=N_CORES)

    xT = nc.dram_tensor("xT", [H, T], F32, kind="ExternalInput")
    # per-core gate slice: core r gets xT[:, r*T/8:(r+1)*T/8] (host-sliced)
    xg_d = nc.dram_tensor("xg", [H, T // N_CORES], F32, kind="ExternalInput")
    gwT = nc.dram_tensor("gwT", [H, E], F32, kind="ExternalInput")
    ident = nc.dram_tensor("ident", [128, 128], F32, kind="ExternalInput")
    wg = nc.dram_tensor("wg", [H, I_R], BF16, kind="ExternalInput")
    wu = nc.dram_tensor("wu", [H, I_R], BF16, kind="ExternalInput")
    wd = nc.dram_tensor("wd", [I_R, H], BF16, kind="ExternalInput")
    swg = nc.dram_tensor("swg", [H, SI], BF16, kind="ExternalInput")
    swu = nc.dram_tensor("swu", [H, SI], BF16, kind="ExternalInput")
    swd = nc.dram_tensor("swd", [SI, H], BF16, kind="ExternalInput")
    y = nc.dram_tensor("y", [128, T], F32, kind="ExternalOutput")

    rg = [list(range(N_CORES))]

    with tile.TileContext(nc, trace_sim=trace_sim) as tc:
        with (
            tc.tile_pool(name="const", bufs=1) as cpool,
            tc.tile_pool(name="xf", bufs=2) as xfpool,
            tc.tile_pool(name="xb", bufs=2) as xbpool,
            tc.tile_pool(name="gate", bufs=2) as gpool,
            tc.tile_pool(name="actr", bufs=2) as actrpool,
            tc.tile_pool(name="acts", bufs=2) as actspool,
            tc.tile_pool(name="tmp", bufs=3) as tpool,
            tc.tile_pool(name="eo", bufs=3) as eopool,
            tc.tile_pool(name="ps_small", bufs=3, space="PSUM") as ps_small,
            tc.tile_pool(name="ps_up", bufs=3, space="PSUM") as ps_up,
            tc.tile_pool(name="ps_o", bufs=2, space="PSUM") as ps_o,
            tc.tile_pool(name="dram", bufs=2, space="DRAM") as dpool,
        ):
            # ---- chunk-0 x + gate weights FIRST so PE starts early ----
            xf0 = xfpool.tile([128, KC, TC], F32, tag="xf")
            for k in range(KC):
                nc.sync.dma_start(xf0[:, k, :], xT[k * 128:(k + 1) * 128, 0:TC])
            gw_t = cpool.tile([128, KC, E], F32)
            for k in range(KC):
                nc.sync.dma_start(gw_t[:, k, :], gwT[k * 128:(k + 1) * 128, :])
            id_t = cpool.tile([128, 128], F32)
            nc.sync.dma_start(id_t[:, :], ident[:, :])
            ones = cpool.tile([1, 128], F32)
            nc.vector.memset(ones[:, :], 1.0)

            # ---- weights, split per contraction chunk so the first
            # up-proj matmuls only wait for their own slice ----
            wg_ks, wu_ks = [], []
            for k in range(KC):
                wgk = cpool.tile([128, I_R], BF16, tag=f"wg{k}")
                nc.sync.dma_start(wgk[:, :], wg[k * 128:(k + 1) * 128, :])
                wuk = cpool.tile([128, I_R], BF16, tag=f"wu{k}")
                nc.sync.dma_start(wuk[:, :], wu[k * 128:(k + 1) * 128, :])
                wg_ks.append(wgk)
                wu_ks.append(wuk)
            swg_ks, swu_ks = [], []
            for k in range(KC):
                sgk = cpool.tile([128, SI], BF16, tag=f"sg{k}")
                nc.sync.dma_start(sgk[:, :], swg[k * 128:(k + 1) * 128, :])
                suk = cpool.tile([128, SI], BF16, tag=f"su{k}")
                nc.sync.dma_start(suk[:, :], swu[k * 128:(k + 1) * 128, :])
                swg_ks.append(sgk)
                swu_ks.append(suk)
            wd_ts = []
            for it in range(IT_R):
                wdt = cpool.tile([128, H], BF16, tag=f"wd{it}")
                nc.sync.dma_start(wdt[:, :], wd[it * 128:(it + 1) * 128, :])
                wd_ts.append(wdt)
            swd_ts = []
            for it, m0, msz in SH_TILES:
                sdt = cpool.tile([128, H], BF16, tag=f"sd{it}")
                nc.sync.dma_start(sdt[:msz, :], swd[m0:m0 + msz, :])
                swd_ts.append(sdt)

            # ---- gate (sharded): each core computes the top-2 softmax
            # weights of ALL experts for ITS T/8-token slice, then one tiny
            # AllToAll redistributes so every core holds its OWN expert's
            # weight for ALL tokens, ordered by token (= chunk-major).
            GT = T // N_CORES
            a2a_in = dpool.tile([E, GT], F32, tag="a2ain")
            a2a_out = dpool.tile([E, GT], F32, tag="a2aout")
            n_gsub = (GT + 127) // 128
            wrow_all = gpool.tile([E, GT], F32, tag="wra")
            for j in range(n_gsub):
                g0 = j * 128
                gsz = min(128, GT - g0)
                xgt = gpool.tile([128, KC, 128], F32, tag="xgt")
                for k in range(KC):
                    nc.sync.dma_start(
                        xgt[:, k, :gsz], xg_d[k * 128:(k + 1) * 128,
                                              g0:g0 + gsz])
                pl = ps_small.tile([128, E], F32, tag="sm")
                for k in range(KC):
                    nc.tensor.matmul(
                        pl[:gsz, :], xgt[:, k, :gsz], gw_t[:, k, :],
                        start=(k == 0), stop=(k == KC - 1))
                lg = gpool.tile([128, E], F32, tag="lg")
                nc.vector.tensor_copy(lg[:gsz, :], pl[:gsz, :])
                m1 = gpool.tile([128, 1], F32, tag="m1")
                nc.vector.reduce_max(m1[:gsz, :], lg[:gsz, :], axis=AX.X)
                eq1 = gpool.tile([128, E], F32, tag="eq1")
                nc.vector.tensor_scalar(
                    eq1[:gsz, :], lg[:gsz, :], m1[:gsz, 0:1], None,
                    op0=ALU.is_equal)
                masked = gpool.tile([128, E], F32, tag="mk")
                nc.vector.scalar_tensor_tensor(
                    masked[:gsz, :], eq1[:gsz, :], NEG_BIG, lg[:gsz, :],
                    op0=ALU.mult, op1=ALU.add)
                m2l = gpool.tile([128, 1], F32, tag="m2l")
                nc.vector.reduce_max(m2l[:gsz, :], masked[:gsz, :], axis=AX.X)
                # w[:, e] = 1[l_e >= m2l] * sigmoid(2*l_e - m1 - m2l)
                arg = gpool.tile([128, E], F32, tag="arg")
                nc.vector.tensor_scalar_mul(arg[:gsz, :], lg[:gsz, :], 2.0)
                nc.vector.tensor_scalar(
                    arg[:gsz, :], arg[:gsz, :], m1[:gsz, 0:1], m2l[:gsz, 0:1],
                    op0=ALU.subtract, op1=ALU.subtract)
                sig = gpool.tile([128, E], F32, tag="sig")
                nc.scalar.activation(sig[:gsz, :], arg[:gsz, :], ACTF.Sigmoid)
                sel = gpool.tile([128, E], F32, tag="sel")
                nc.vector.tensor_scalar(
                    sel[:gsz, :], lg[:gsz, :], m2l[:gsz, 0:1], None,
                    op0=ALU.is_ge)
                wcol = gpool.tile([128, E], F32, tag="wc")
                nc.vector.tensor_mul(wcol[:gsz, :], sig[:gsz, :], sel[:gsz, :])
                ptr = ps_small.tile([E, 128], F32, tag="sm")
                nc.tensor.transpose(ptr[:, :gsz], wcol[:gsz, :],
                                    id_t[:gsz, :gsz])
                nc.vector.tensor_copy(wrow_all[:, g0:g0 + gsz], ptr[:, :gsz])
            nc.sync.dma_start(a2a_in[:, :], wrow_all[:, :])
            nc.gpsimd.collective_compute(
                "AllToAll", ALU.bypass, replica_groups=rg,
                ins=[a2a_in.opt()], outs=[a2a_out.opt()])
            # row-major element t of a2a_out is this expert's weight for
            # global token t

            for c in range(n_chunks):
                t0 = c * TC
                # ---- load x chunk (f32) and cast to bf16 ----
                if c == 0:
                    xf = xf0
                else:
                    xf = xfpool.tile([128, KC, TC], F32, tag="xf")
                    for k in range(KC):
                        nc.sync.dma_start(
                            xf[:, k, :], xT[k * 128:(k + 1) * 128, t0:t0 + TC])
                xb = xbpool.tile([128, KC, TC], BF16)
                nc.vector.tensor_copy(xb[:, :, :], xf[:, :, :])

                # ---- gate weight row for this chunk (from AllToAll) ----
                wrow = gpool.tile([1, TC], F32)
                if GT >= TC:
                    r0 = t0 // GT
                    o0 = t0 % GT
                    nc.sync.dma_start(
                        wrow[0:1, :], a2a_out[r0:r0 + 1, o0:o0 + TC])
                else:
                    for b in range(TC // GT):
                        r0 = (t0 + b * GT) // GT
                        nc.sync.dma_start(
                            wrow[0:1, b * GT:(b + 1) * GT],
                            a2a_out[r0:r0 + 1, :])
                # broadcast w over 128 partitions
                pw = ps_small.tile([128, TC], F32, tag="sm")
                nc.tensor.matmul(pw[:, :], ones[0:1, :], wrow[0:1, :],
                                 start=True, stop=True)
                wb = gpool.tile([128, TC], F32)
                nc.vector.tensor_copy(wb[:, :], pw[:, :])

                # ---- routed expert up-proj + swiglu (scaled by gate w) ----
                actr = actrpool.tile([128, IT_R, TC], BF16)
                for it in range(IT_R):
                    pg = ps_up.tile([128, TC], F32, tag="up")
                    for k in range(KC):
                        nc.tensor.matmul(
                            pg[:, :], wg_ks[k][:, it * 128:(it + 1) * 128],
                            xb[:, k, :], start=(k == 0), stop=(k == KC - 1))
                    pu = ps_up.tile([128, TC], F32, tag="up")
                    for k in range(KC):
                        nc.tensor.matmul(
                            pu[:, :], wu_ks[k][:, it * 128:(it + 1) * 128],
                            xb[:, k, :], start=(k == 0), stop=(k == KC - 1))
                    sg = tpool.tile([128, TC], F32, tag="sg")
                    if silu_via_sigmoid:
                        nc.scalar.activation(sg[:, :], pg[:, :], ACTF.Sigmoid)
                        nc.vector.tensor_mul(sg[:, :], sg[:, :], pg[:, :])
                    else:
                        nc.scalar.activation(sg[:, :], pg[:, :], ACTF.Silu)
                    tt = tpool.tile([128, TC], F32, tag="tt")
                    nc.vector.tensor_mul(tt[:, :], sg[:, :], pu[:, :])
                    nc.vector.tensor_mul(actr[:, it, :], tt[:, :], wb[:, :])

                # ---- shared expert shard up-proj + swiglu ----
                acts = actspool.tile([128, len(SH_TILES), TC], BF16)
                for it, m0, msz in SH_TILES:
                    pg = ps_up.tile([128, TC], F32, tag="up")
                    for k in range(KC):
                        nc.tensor.matmul(
                            pg[:msz, :], swg_ks[k][:, m0:m0 + msz],
                            xb[:, k, :], start=(k == 0), stop=(k == KC - 1))
                    pu = ps_up.tile([128, TC], F32, tag="up")
                    for k in range(KC):
                        nc.tensor.matmul(
                            pu[:msz, :], swu_ks[k][:, m0:m0 + msz],
                            xb[:, k, :], start=(k == 0), stop=(k == KC - 1))
                    sg = tpool.tile([128, TC], F32, tag="sg")
                    if silu_via_sigmoid:
                        nc.scalar.activation(sg[:msz, :], pg[:msz, :],
                                             ACTF.Sigmoid)
                        nc.vector.tensor_mul(sg[:msz, :], sg[:msz, :],
                                             pg[:msz, :])
                    else:
                        nc.scalar.activation(sg[:msz, :], pg[:msz, :],
                                             ACTF.Silu)
                    nc.vector.tensor_mul(acts[:msz, it, :], sg[:msz, :],
                                         pu[:msz, :])

                # ---- down-proj (routed + shared into one accumulator) ----
                ccin = dpool.tile([H, TC], F32, tag="ccin")
                for hc in range(KC):
                    h0 = hc * 128
                    po = ps_o.tile([128, TC], F32, tag="o")
                    for it in range(IT_R):
                        nc.tensor.matmul(
                            po[:, :], wd_ts[it][:, h0:h0 + 128],
                            actr[:, it, :], start=(it == 0), stop=False)
                    for it, m0, msz in SH_TILES:
                        nc.tensor.matmul(
                            po[:, :], swd_ts[it][:msz, h0:h0 + 128],
                            acts[:msz, it, :], start=False,
                            stop=(it == len(SH_TILES) - 1))
                    eo = eopool.tile([128, TC], F32)
                    nc.vector.tensor_copy(eo[:, :], po[:, :])
                    nc.sync.dma_start(ccin[h0:h0 + 128, :], eo[:, :])

                # ---- combine across cores: ReduceScatter this chunk ----
                ccout = dpool.tile([128, TC], F32, tag="ccout")
                nc.gpsimd.collective_compute(
                    "ReduceScatter", ALU.add, replica_groups=rg,
                    ins=[ccin.opt()], outs=[ccout.opt()])
                nc.sync.dma_start(y[:, t0:t0 + TC], ccout[:, :])

    nc.compile()
    return nc


def make_in_maps(x, gate_w, wg, wu, wd, swg, swu, swd, T=4096):
    xT = np.ascontiguousarray(
        x.reshape(-1, H).T).astype(np.float32)[:, :T]
    ident = np.eye(128, dtype=np.float32)
    in_maps = []
    GT = T // N_CORES
    gwT_g = np.ascontiguousarray(gate_w.T.astype(np.float32))
    for r in range(N_CORES):
        in_maps.append({
            "xT": xT,
            "xg": np.ascontiguousarray(xT[:, r * GT:(r + 1) * GT]),
            "gwT": gwT_g,
            "ident": ident,
            "wg": np.ascontiguousarray(wg[r]).astype(BF16_NP),
            "wu": np.ascontiguousarray(wu[r]).astype(BF16_NP),
            "wd": np.ascontiguousarray(wd[r]).astype(BF16_NP),
            "swg": np.ascontiguousarray(swg[:, r * SI:(r + 1) * SI]).astype(BF16_NP),
            "swu": np.ascontiguousarray(swu[:, r * SI:(r + 1) * SI]).astype(BF16_NP),
            "swd": np.ascontiguousarray(swd[r * SI:(r + 1) * SI, :]).astype(BF16_NP),
        })
    return in_maps


_NC_CACHE = {}


def kernel(x, gate_w, wg, wu, wd, swg, swu, swd):
    global LAST_RESULT
    x = np.asarray(x)
    B, S, _ = x.shape
    T = B * S
    if T not in _NC_CACHE:
        _NC_CACHE[T] = build_nc(T=T)
    nc = _NC_CACHE[T]
    in_maps = make_in_maps(
        np.asarray(x, np.float32), np.asarray(gate_w, np.float32),
        np.asarray(wg, np.float32), np.asarray(wu, np.float32),
        np.asarray(wd, np.float32), np.asarray(swg, np.float32),
        np.asarray(swu, np.float32), np.asarray(swd, np.float32), T=T)
    res = run_bass_kernel_spmd(nc, in_maps, core_ids=list(range(N_CORES)))
    LAST_RESULT = res
    yT = np.concatenate([res.results[r]["y"] for r in range(N_CORES)], axis=0)
    return np.ascontiguousarray(yT.T).reshape(B, S, H).astype(np.float32)
